# revision 15
# baseline (speedup 1.0000x reference)
"""Bass/Trainium2 kernel for Kimi-style MLA attention (nn_KimiMLAAttention).

Strategy (8 NeuronCores, tensor-parallel over heads):
  - 16 heads -> 2 heads per core. Each core computes q-projection for its 2
    heads, the (replicated) compressed-kv projection + rmsnorm, per-head
    k-embed / v-unembed from the shared latent, causal attention in a
    TRANSPOSED score layout (scores^T[s, l]), and a partial o_proj against
    its 2-head slice of Wo. Host sums the 8 partial outputs.

v2 performance notes (from the v1 trace):
  - All matmul operands are bf16 (PSUM accumulation stays fp32). Same PE
    rate as fp32r but half the DMA/SBUF traffic and shorter weight loads.
  - The PE clock is HAM-gated: any PE-idle gap re-throttles it to 1.2 GHz.
    v1 lost ~240us to oscillation because softmax/rmsnorm tails (DVE
    reciprocal -> broadcast matmul) sat in the PE FIFO. Here every slow
    tail is software-pipelined: its PE matmuls are emitted only after the
    NEXT compute burst, so the reciprocal runs concurrently with matmuls.
  - Causal diagonal s-tiles use trimmed moving dims (columns >= 128*d) and
    a single 128x128 triangular mask multiply.
"""

from contextlib import ExitStack

import numpy as np
import ml_dtypes

import concourse.bass as bass
import concourse.tile as tile
from concourse import mybir
from concourse.bass import ds, ts
from concourse.bass_utils import run_bass_kernel_spmd

F32 = mybir.dt.float32
F32R = mybir.dt.float32r
BF16 = mybir.dt.bfloat16
AF = mybir.ActivationFunctionType
NPBF16 = ml_dtypes.bfloat16


def _patch_tile_tail_drain():
    """walrus's CoreV3 codegen rejects the TileContext tail drain when it
    carries >1 sem waits ("Too many sync wait commands"). Split the waits
    across multiple single-wait drain instructions on the sync engine."""
    if getattr(tile.TileContext, "_tail_drain_patched", False):
        return
    from concourse.vector_clock import ScopedClock

    def _drain_and_barrier(self, tick_clock, wait_clock):
        nc = self.nc
        drain_inst = nc.sync.drain()
        wait_clock.add_sem_waits(
            drain_inst.ins, ScopedClock({None: tick_clock.global_clock})
        )
        inst = drain_inst.ins
        si = inst.sync_info
        if si is not None and si.on_wait is not None and len(si.on_wait) > 1:
            waits = list(si.on_wait)
            upd = list(si.on_update) if si.on_update else []
            inst.sync_info = mybir.SyncInfo(on_wait=waits[:1], on_update=[])
            for i, w in enumerate(waits[1:]):
                extra = nc.sync.drain()
                last = i == len(waits) - 2
                extra.ins.sync_info = mybir.SyncInfo(
                    on_wait=[w], on_update=upd if last else []
                )
        nc.all_engine_barrier()
        assert self.sems is not None
        popped = nc._tile_sem_poison_stack.pop()
        assert popped is self._sem_poison
        nc.clear_and_free_semaphores(list(self.sems.allocated().values()))
        nc.all_engine_barrier()

    tile.TileContext._drain_and_barrier = _drain_and_barrier
    tile.TileContext._tail_drain_patched = True


_patch_tile_tail_drain()


def _split_excess_waits(nc, max_waits=1):
    """walrus's per-instruction sync-wait slots are tiny on this compiler
    build; hoist excess sem waits onto same-engine NoOp carriers placed
    immediately before the instruction (waits fire earlier in the same
    engine stream, so ordering semantics are preserved)."""
    for f in nc.m.functions:
        for bb in f.blocks:
            insts = bb.instructions
            if not any(
                i.sync_info is not None
                and i.sync_info.on_wait
                and len(i.sync_info.on_wait) > max_waits
                for i in insts
            ):
                continue
            out = []
            for inst in insts:
                si = inst.sync_info
                if si is not None and si.on_wait and len(si.on_wait) > max_waits:
                    waits = list(si.on_wait)
                    for w in waits[:-max_waits]:
                        nop = mybir.InstNoOp(
                            name=nc.get_next_instruction_name(), ins=[], outs=[]
                        )
                        nop.engine = inst.engine
                        nop.sync_info = mybir.SyncInfo(on_wait=[w], on_update=[])
                        out.append(nop)
                    inst.sync_info = mybir.SyncInfo(
                        on_wait=waits[-max_waits:],
                        on_update=list(si.on_update) if si.on_update else [],
                    )
                out.append(inst)
            bb.instructions = out


B, L, HID = 1, 2048, 2048
H = 16
NOPE, ROPE, VDIM, LORA = 128, 64, 128, 512
QDIM = NOPE + ROPE
EPS = 1e-5
SCALE = QDIM**-0.5
NCORES = 8
HPC = H // NCORES  # 2 heads per core

LCH = 512  # moving-operand chunk (max moving free dim / PSUM bank)
NJ = L // LCH  # 4 l-chunks
NK = HID // 128  # 16 contraction tiles for projections
NS = L // 128  # 16 s(key)-tiles
NLAT = LORA // 128  # 4 latent partition tiles
WCOLS = 960  # fused projection weight columns
# m-chunks of wqkv columns: h0 nope, h1 nope, ropes, 4x latent, k_pe
MS = [(0, 128), (128, 128), (256, 128), (384, 128), (512, 128),
      (640, 128), (768, 128), (896, 64)]


def _build_nc():
    nc = bass.Bass()
    xT = nc.dram_tensor("xT", [HID, L], BF16, kind="ExternalInput")
    wqkv = nc.dram_tensor("wqkv", [HID, WCOLS], BF16, kind="ExternalInput")
    we = nc.dram_tensor("we", [HPC, LORA, NOPE], BF16, kind="ExternalInput")
    wu = nc.dram_tensor("wu", [LORA, HPC * VDIM], BF16, kind="ExternalInput")
    wo0 = nc.dram_tensor("wo0", [VDIM, HID], BF16, kind="ExternalInput")
    wo1 = nc.dram_tensor("wo1", [VDIM, HID], BF16, kind="ExternalInput")
    mtri_d = nc.dram_tensor("mtri", [128, 128], BF16, kind="ExternalInput")
    ones_col_d = nc.dram_tensor("ones_col_d", [128, 1], BF16, kind="ExternalInput")
    ones_row_d = nc.dram_tensor("ones_row_d", [1, 128], BF16, kind="ExternalInput")
    y = nc.dram_tensor("y", [L, HID], BF16, kind="ExternalOutput")

    mm = nc.tensor.matmul

    with tile.TileContext(nc) as tc, ExitStack() as ctx:
        persist = ctx.enter_context(tc.tile_pool(name="persist", bufs=1))
        qn = [persist.tile([128, L], BF16, name=f"qn{h}", tag=f"qn{h}") for h in range(HPC)]
        qr = persist.tile([128, L], BF16, name="qr", tag="qr")
        kpe = [persist.tile([128, L], BF16, name=f"kpe{h}", tag=f"kpe{h}")
               for h in range(HPC)]
        latT = [persist.tile([128, L], BF16, name=f"latT{i}", tag=f"latT{i}") for i in range(NLAT)]
        kT = [persist.tile([128, L], BF16, name=f"kT{h}", tag=f"kT{h}") for h in range(HPC)]
        outT = [persist.tile([128, L], BF16, name=f"outT{h}", tag=f"outT{h}") for h in range(HPC)]
        vsb = persist.tile([128, NS * HPC * VDIM], BF16, name="vsb", tag="vsb")
        mtri = persist.tile([128, 128], BF16, name="mtri_sb", tag="mtri_sb")
        ones_col = persist.tile([128, 1], BF16, name="ones_col", tag="ones_col")
        ones_row = persist.tile([1, 128], BF16, name="ones_row", tag="ones_row")
        eps_col = persist.tile([1, 1], F32, name="eps_col", tag="eps_col")
        w_sb = [persist.tile([128, WCOLS], BF16, name=f"w{k}", tag=f"w{k}") for k in range(NK)]
        x_sb = [persist.tile([128, L], BF16, name=f"x{k}", tag=f"x{k}") for k in range(NK)]
        we_sb = [[persist.tile([128, NOPE], BF16, name=f"we{h}{i}", tag=f"we{h}{i}")
                  for i in range(NLAT)] for h in range(HPC)]
        wu_sb = [persist.tile([128, HPC * VDIM], BF16, name=f"wu{i}", tag=f"wu{i}")
                 for i in range(NLAT)]
        wo_sb = [persist.tile([128, HID], BF16, name=f"wo{hh}", tag=f"wo{hh}")
                 for hh in range(HPC)]
        rows = ctx.enter_context(tc.tile_pool(name="rows", bufs=2))
        sqp = ctx.enter_context(tc.tile_pool(name="sqp", bufs=1))

        nc.vector.memset(eps_col, EPS)
        nc.vector.memset(kpe[0][64:128, :], 0.0)
        nc.vector.memset(kpe[1][0:64, :], 0.0)

        # ---------------- P0: fused projections + pipelined rmsnorm ----------
        p0_stack = ExitStack()
        pp0 = p0_stack.enter_context(tc.tile_pool(name="pp0", bufs=1, space="PSUM"))

        def p0_mm(j):
            pss = [pp0.tile([128, LCH], F32, name=f"pm{m}", tag=f"pm{m}") for m in range(8)]
            for k in range(NK):
                if j == 0:
                    nc.sync.dma_start(out=w_sb[k], in_=wqkv[ts(k, 128), :])
                    nc.sync.dma_start(out=x_sb[k][:, 0 : 2 * LCH],
                                      in_=xT[ts(k, 128), 0 : 2 * LCH])
                if j == 1:
                    nc.sync.dma_start(out=x_sb[k][:, 2 * LCH : L],
                                      in_=xT[ts(k, 128), 2 * LCH : L])
                if j == 0 and k == 3:
                    nc.sync.dma_start(out=ones_col, in_=ones_col_d[:, :])
                    nc.sync.dma_start(out=ones_row, in_=ones_row_d[:, :])
                if j == 0 and k == 6:
                    nc.sync.dma_start(out=mtri, in_=mtri_d[:, :])
                xt = x_sb[k][:, ts(j, LCH)]
                for m, (c0, cw) in enumerate(MS):
                    mm(pss[m][:cw, :], (w_sb[k][:, ds(c0, cw)]), (xt),
                       start=(k == 0), stop=(k == NK - 1))
            return pss

        def p0_copy(j, pss):
            jc = ds(j * LCH, LCH)
            order = list(range(8)) if j == 0 else [7, 0, 1, 2, 3, 4, 5, 6]
            with nc.allow_low_precision(reason="bf16 activations"):
                for m in order:
                    if m == 7:
                        nc.vector.tensor_copy(kpe[0][0:64, jc], pss[7][0:64, :])
                        nc.sync.dma_start(out=kpe[1][64:128, jc],
                                          in_=kpe[0][0:64, jc])
                    elif m == 0:
                        nc.vector.tensor_copy(qn[0][:, jc], pss[0])
                    elif m == 1:
                        nc.vector.tensor_copy(qn[1][:, jc], pss[1])
                    elif m == 2:
                        nc.vector.tensor_copy(qr[:, jc], pss[2])
                    else:
                        nc.vector.tensor_copy(latT[m - 3][:, jc], pss[m])

        def p0_sqmul(j):
            jc = ds(j * LCH, LCH)
            sqs = [sqp.tile([128, LCH], BF16, name=f"sq{i}", tag=f"sq{i}")
                   for i in range(NLAT)]
            with nc.allow_low_precision(reason="bf16 squares"):
                for i in range(NLAT):
                    nc.vector.tensor_mul(sqs[i], latT[i][:, jc], latT[i][:, jc])
            return sqs

        def p0_ssq(j, sqs, pool, tag):
            ssq = pool.tile([1, LCH], F32, name="ssq", tag=tag)
            for i in range(NLAT):
                mm(ssq, (ones_col), (sqs[i]), start=(i == 0), stop=(i == NLAT - 1))
            ln_row = rows.tile([1, LCH], F32, name="ln_row", tag="lnrow")
            nc.scalar.activation(ln_row, ssq, AF.Ln, bias=eps_col[0:1, :],
                                 scale=1.0 / LORA)
            scale_row = rows.tile([1, LCH], BF16, name="scale_row", tag="scrow")
            with nc.allow_low_precision(reason="bf16 row for broadcast matmul"):
                nc.scalar.activation(scale_row, ln_row, AF.Exp, scale=-0.5)
            return scale_row

        def p0_norm(j, scale_row, pool, tag):
            jc = ds(j * LCH, LCH)
            bc = pool.tile([128, LCH], F32, name="bc", tag=tag)
            mm(bc, (ones_row), (scale_row), start=True, stop=True)
            with nc.allow_low_precision(reason="bf16 normalized latent"):
                for i in range(NLAT):
                    nc.vector.tensor_mul(latT[i][:, jc], latT[i][:, jc], bc)

        pss_h, sqs_h, row_h = {}, {}, {}
        for j in range(NJ):
            if j >= 2:
                p0_norm(j - 2, row_h[j - 2], pp0, "pm6")
            pss_h[j] = p0_mm(j)
            if j == 2:  # prefetch P2 weights behind burst-2's xt queue
                for h in range(HPC):
                    for i in range(NLAT):
                        nc.sync.dma_start(out=we_sb[h][i], in_=we[h, ts(i, 128), :])
                for i in range(NLAT):
                    nc.sync.dma_start(out=wu_sb[i], in_=wu[ts(i, 128), :])
            if j >= 1:
                sqs_h[j - 1] = p0_sqmul(j - 1)
            p0_copy(j, pss_h[j])
            if j >= 1:
                row_h[j - 1] = p0_ssq(j - 1, sqs_h[j - 1], pp0, "pm7")
        nc.sync.dma_start(out=wo_sb[0], in_=wo0[:, :])
        nc.sync.dma_start(out=wo_sb[1], in_=wo1[:, :])
        sqs_h[3] = p0_sqmul(3)
        p0_stack.close()

        # ---------------- P2: k/v embed, interleaved with rmsnorm finish -----
        p2_stack = ExitStack()
        pp2 = p2_stack.enter_context(tc.tile_pool(name="pp2", bufs=1, space="PSUM"))

        def p2_pv(si):
            pv = pp2.tile([128, HPC * VDIM], F32, name="pv", tag="pv", bufs=2)
            for i in range(NLAT):
                mm(pv, (latT[i][:, ts(si, 128)]), (wu_sb[i]),
                   start=(i == 0), stop=(i == NLAT - 1))
            with nc.allow_low_precision(reason="bf16 v"):
                nc.vector.tensor_copy(vsb[:, ds(si * HPC * VDIM, HPC * VDIM)], pv)

        def p2_kt(h, j):
            pk = pp2.tile([128, LCH], F32, name="pk", tag="pk", bufs=2)
            for i in range(NLAT):
                mm(pk, (we_sb[h][i]), (latT[i][:, ts(j, LCH)]),
                   start=(i == 0), stop=(i == NLAT - 1))
            with nc.allow_low_precision(reason="bf16 k"):
                nc.vector.tensor_copy(kT[h][:, ts(j, LCH)], pk)

        for si in range(8):
            p2_pv(si)
        row_h[3] = p0_ssq(3, sqs_h[3], pp2, "ssqx")
        p0_norm(2, row_h[2], pp2, "bcx")
        for si in range(8, 12):
            p2_pv(si)
        p2_kt(0, 0)
        p2_kt(1, 0)
        p0_norm(3, row_h[3], pp2, "bcx")
        for h in range(HPC):
            p2_kt(h, 1)
            p2_kt(h, 2)
        for si in range(12, 16):
            p2_pv(si)
        p2_kt(0, 3)
        p2_kt(1, 3)
        p2_stack.close()

        # ---------------- P3: causal attention (pipelined tails) + P4 --------
        with (
            tc.tile_pool(name="pp3", bufs=1, space="PSUM") as pp3,
            tc.tile_pool(name="epool", bufs=4) as epool,
            tc.tile_pool(name="ypool", bufs=2) as ypool,
            tc.tile_pool(name="bpool", bufs=2) as bpool,
        ):
            def attn_burst(j, h):
                nsi = 4 * j + 4
                jc0 = j * LCH
                pcs = pp3.tile([1, LCH], F32, name="pcs", tag="pcs", bufs=2)
                po = pp3.tile([128, LCH], F32, name="po", tag="po", bufs=2)
                pend = []

                def flush_one():
                    si2, c2, w2, e2 = pend.pop(0)
                    mm(pcs[:, ds(c2, w2)], (ones_col), (e2[:, ds(c2, w2)]),
                       start=(si2 == 0), stop=(si2 == nsi - 1))
                    mm(po[:, ds(c2, w2)],
                       (vsb[:, ds(si2 * HPC * VDIM + h * VDIM, VDIM)]),
                       (e2[:, ds(c2, w2)]),
                       start=(si2 == 0), stop=(si2 == nsi - 1))

                for si in range(nsi):
                    d = si - 4 * j
                    c0 = 128 * d if d >= 0 else 0
                    w = LCH - c0
                    ps = pp3.tile([128, LCH], F32, name="ps", tag="ps", bufs=3)
                    mm(ps[:, ds(c0, w)], (kT[h][:, ts(si, 128)]),
                       (qn[h][:, ds(jc0 + c0, w)]), start=True, stop=False)
                    mm(ps[:, ds(c0, w)], (kpe[h][:, ts(si, 128)]),
                       (qr[:, ds(jc0 + c0, w)]), start=False, stop=True)
                    e = epool.tile([128, LCH], BF16, name="e", tag="e")
                    with nc.allow_low_precision(reason="bf16 attn weights"):
                        nc.scalar.activation(e[:, ds(c0, w)], ps[:, ds(c0, w)],
                                             AF.Exp, scale=SCALE)
                        if d >= 0:
                            nc.vector.tensor_mul(e[:, ds(c0, 128)],
                                                 e[:, ds(c0, 128)], mtri)
                    pend.append((si, c0, w, e))
                    if len(pend) > 2:
                        flush_one()
                while pend:
                    flush_one()
                return pcs, po

            def attn_tail(j, h, pcs, po):
                lnr = rows.tile([1, LCH], F32, name="lnr", tag="lnr")
                nc.scalar.activation(lnr, pcs, AF.Ln)
                rrow = rows.tile([1, LCH], BF16, name="rrow", tag="rrow")
                with nc.allow_low_precision(reason="bf16 row for broadcast matmul"):
                    nc.scalar.activation(rrow, lnr, AF.Exp, scale=-1.0)
                pbc = pp3.tile([128, LCH], F32, name="pbc", tag="pbc", bufs=1)
                mm(pbc, (ones_row), (rrow), start=True, stop=True)
                bcs = bpool.tile([128, LCH], BF16, name="bcs", tag="bcs")
                with nc.allow_low_precision(reason="bf16 attn output"):
                    nc.vector.tensor_copy(bcs, pbc)
                    nc.vector.tensor_mul(outT[h][:, ts(j, LCH)], po, bcs)

            prev = None
            for j in (3, 2, 1, 0):  # longest bursts first: warm HAM early
                for h in range(HPC):
                    cur = (j, h) + attn_burst(j, h)
                    if prev is not None:
                        attn_tail(*prev)
                    prev = cur
            attn_tail(*prev)

            # ---- P4: partial o_proj y = outT.T @ Wo[2-head rows] ----
            for i in range(NS):
                ysb = ypool.tile([128, HID], BF16, name="ysb", tag="ysb")
                for n in range(NJ):
                    py = pp3.tile([128, LCH], F32, name="py", tag="ps", bufs=3)
                    mm(py, (outT[0][:, ts(i, 128)]), (wo_sb[0][:, ts(n, LCH)]),
                       start=True, stop=False)
                    mm(py, (outT[1][:, ts(i, 128)]), (wo_sb[1][:, ts(n, LCH)]),
                       start=False, stop=True)
                    with nc.allow_low_precision(reason="bf16 partial output"):
                        nc.vector.tensor_copy(
                            ysb[:, ds(n * LCH, 256)], py[:, 0:256])
                        nc.scalar.copy(
                            ysb[:, ds(n * LCH + 256, 256)], py[:, 256:512])
                nc.sync.dma_start(out=y[ts(i, 128), :], in_=ysb)

    _split_excess_waits(nc)
    return nc


_NC_CACHE = None


def _get_nc():
    global _NC_CACHE
    if _NC_CACHE is None:
        _NC_CACHE = _build_nc()
    return _NC_CACHE


def _make_in_maps(x, Wq, Wkv_a, kv_ln_w, W_embed, W_unembed, Wo):
    xT = np.ascontiguousarray(
        np.asarray(x, dtype=np.float32)[0].T).astype(NPBF16)
    Wq = np.asarray(Wq, dtype=np.float32)
    Wkv_a = np.asarray(Wkv_a, dtype=np.float32)
    kv_ln_w = np.asarray(kv_ln_w, dtype=np.float32)
    W_embed = np.asarray(W_embed, dtype=np.float32)
    W_unembed = np.asarray(W_unembed, dtype=np.float32)
    Wo = np.asarray(Wo, dtype=np.float32)

    Wq3 = Wq.reshape(HID, H, QDIM)
    # triangular diagonal-band mask: mtri[p, c] = 1 iff c >= p
    idx = np.arange(128)
    mtri = (idx[None, :] >= idx[:, None]).astype(NPBF16)

    in_maps = []
    for c in range(NCORES):
        h0, h1 = HPC * c, HPC * c + 1
        wqkv = np.concatenate(
            [
                Wq3[:, h0, :NOPE],
                Wq3[:, h1, :NOPE],
                Wq3[:, h0, NOPE:],
                Wq3[:, h1, NOPE:],
                Wkv_a,
            ],
            axis=1,
        )
        we_ = np.ascontiguousarray(
            W_embed[[h0, h1]] * kv_ln_w[None, :, None]).astype(NPBF16)
        wu_ = np.ascontiguousarray(
            np.concatenate([W_unembed[h0].T, W_unembed[h1].T], axis=1)
            * kv_ln_w[:, None]).astype(NPBF16)
        in_maps.append(
            {
                "xT": xT,
                "wqkv": np.ascontiguousarray(wqkv).astype(NPBF16),
                "we": we_,
                "wu": wu_,
                "wo0": np.ascontiguousarray(
                    Wo[h0 * VDIM: (h0 + 1) * VDIM]).astype(NPBF16),
                "wo1": np.ascontiguousarray(
                    Wo[h1 * VDIM: (h1 + 1) * VDIM]).astype(NPBF16),
                "mtri": mtri,
                "ones_col_d": np.ones((128, 1), NPBF16),
                "ones_row_d": np.ones((1, 128), NPBF16),
            }
        )
    return in_maps


def run(trace=False, tmpdir=None, **inputs):
    """Run the SPMD kernel; returns (full_output, BassKernelResults)."""
    inputs.pop("mask", None)  # causal structure is hardcoded
    nc = _get_nc()
    in_maps = _make_in_maps(**inputs)
    res = run_bass_kernel_spmd(
        nc, in_maps, core_ids=list(range(NCORES)), trace=trace, tmpdir=tmpdir
    )
    y = np.zeros((L, HID), dtype=np.float32)
    for c in range(NCORES):
        y += np.asarray(res.results[c]["y"], dtype=np.float32)
    return y.reshape(B, L, HID), res


def kernel(**inputs):
    y, _ = run(trace=False, **inputs)
    return y


# revision 16
# speedup vs baseline: 1.0167x; 1.0167x over previous
"""Bass/Trainium2 kernel for Kimi-style MLA attention (nn_KimiMLAAttention).

Strategy (8 NeuronCores, tensor-parallel over heads):
  - 16 heads -> 2 heads per core. Each core computes q-projection for its 2
    heads, the (replicated) compressed-kv projection + rmsnorm, per-head
    k-embed / v-unembed from the shared latent, causal attention in a
    TRANSPOSED score layout (scores^T[s, l]), and a partial o_proj against
    its 2-head slice of Wo. Host sums the 8 partial outputs.

v2 performance notes (from the v1 trace):
  - All matmul operands are bf16 (PSUM accumulation stays fp32). Same PE
    rate as fp32r but half the DMA/SBUF traffic and shorter weight loads.
  - The PE clock is HAM-gated: any PE-idle gap re-throttles it to 1.2 GHz.
    v1 lost ~240us to oscillation because softmax/rmsnorm tails (DVE
    reciprocal -> broadcast matmul) sat in the PE FIFO. Here every slow
    tail is software-pipelined: its PE matmuls are emitted only after the
    NEXT compute burst, so the reciprocal runs concurrently with matmuls.
  - Causal diagonal s-tiles use trimmed moving dims (columns >= 128*d) and
    a single 128x128 triangular mask multiply.
"""

from contextlib import ExitStack

import numpy as np
import ml_dtypes

import concourse.bass as bass
import concourse.tile as tile
from concourse import mybir
from concourse.bass import ds, ts
from concourse.bass_utils import run_bass_kernel_spmd

F32 = mybir.dt.float32
F32R = mybir.dt.float32r
BF16 = mybir.dt.bfloat16
AF = mybir.ActivationFunctionType
NPBF16 = ml_dtypes.bfloat16


def _patch_tile_tail_drain():
    """walrus's CoreV3 codegen rejects the TileContext tail drain when it
    carries >1 sem waits ("Too many sync wait commands"). Split the waits
    across multiple single-wait drain instructions on the sync engine."""
    if getattr(tile.TileContext, "_tail_drain_patched", False):
        return
    from concourse.vector_clock import ScopedClock

    def _drain_and_barrier(self, tick_clock, wait_clock):
        nc = self.nc
        drain_inst = nc.sync.drain()
        wait_clock.add_sem_waits(
            drain_inst.ins, ScopedClock({None: tick_clock.global_clock})
        )
        inst = drain_inst.ins
        si = inst.sync_info
        if si is not None and si.on_wait is not None and len(si.on_wait) > 1:
            waits = list(si.on_wait)
            upd = list(si.on_update) if si.on_update else []
            inst.sync_info = mybir.SyncInfo(on_wait=waits[:1], on_update=[])
            for i, w in enumerate(waits[1:]):
                extra = nc.sync.drain()
                last = i == len(waits) - 2
                extra.ins.sync_info = mybir.SyncInfo(
                    on_wait=[w], on_update=upd if last else []
                )
        nc.all_engine_barrier()
        assert self.sems is not None
        popped = nc._tile_sem_poison_stack.pop()
        assert popped is self._sem_poison
        nc.clear_and_free_semaphores(list(self.sems.allocated().values()))
        nc.all_engine_barrier()

    tile.TileContext._drain_and_barrier = _drain_and_barrier
    tile.TileContext._tail_drain_patched = True


_patch_tile_tail_drain()


def _split_excess_waits(nc, max_waits=1):
    """walrus's per-instruction sync-wait slots are tiny on this compiler
    build; hoist excess sem waits onto same-engine NoOp carriers placed
    immediately before the instruction (waits fire earlier in the same
    engine stream, so ordering semantics are preserved)."""
    for f in nc.m.functions:
        for bb in f.blocks:
            insts = bb.instructions
            if not any(
                i.sync_info is not None
                and i.sync_info.on_wait
                and len(i.sync_info.on_wait) > max_waits
                for i in insts
            ):
                continue
            out = []
            for inst in insts:
                si = inst.sync_info
                if si is not None and si.on_wait and len(si.on_wait) > max_waits:
                    waits = list(si.on_wait)
                    for w in waits[:-max_waits]:
                        nop = mybir.InstNoOp(
                            name=nc.get_next_instruction_name(), ins=[], outs=[]
                        )
                        nop.engine = inst.engine
                        nop.sync_info = mybir.SyncInfo(on_wait=[w], on_update=[])
                        out.append(nop)
                    inst.sync_info = mybir.SyncInfo(
                        on_wait=waits[-max_waits:],
                        on_update=list(si.on_update) if si.on_update else [],
                    )
                out.append(inst)
            bb.instructions = out


B, L, HID = 1, 2048, 2048
H = 16
NOPE, ROPE, VDIM, LORA = 128, 64, 128, 512
QDIM = NOPE + ROPE
EPS = 1e-5
SCALE = QDIM**-0.5
NCORES = 8
HPC = H // NCORES  # 2 heads per core

LCH = 512  # moving-operand chunk (max moving free dim / PSUM bank)
NJ = L // LCH  # 4 l-chunks
NK = HID // 128  # 16 contraction tiles for projections
NS = L // 128  # 16 s(key)-tiles
NLAT = LORA // 128  # 4 latent partition tiles
WCOLS = 960  # fused projection weight columns
# m-chunks of wqkv columns: h0 nope, h1 nope, ropes, 4x latent, k_pe
MS = [(0, 128), (128, 128), (256, 128), (384, 128), (512, 128),
      (640, 128), (768, 128), (896, 64)]


def _build_nc():
    nc = bass.Bass()
    xT = nc.dram_tensor("xT", [HID, L], BF16, kind="ExternalInput")
    wqkv = nc.dram_tensor("wqkv", [HID, WCOLS], BF16, kind="ExternalInput")
    we = nc.dram_tensor("we", [HPC, LORA, NOPE], BF16, kind="ExternalInput")
    wu = nc.dram_tensor("wu", [LORA, HPC * VDIM], BF16, kind="ExternalInput")
    wo0 = nc.dram_tensor("wo0", [VDIM, HID], BF16, kind="ExternalInput")
    wo1 = nc.dram_tensor("wo1", [VDIM, HID], BF16, kind="ExternalInput")
    mtri_d = nc.dram_tensor("mtri", [128, 128], BF16, kind="ExternalInput")
    ones_col_d = nc.dram_tensor("ones_col_d", [128, 1], BF16, kind="ExternalInput")
    ones_row_d = nc.dram_tensor("ones_row_d", [1, 128], BF16, kind="ExternalInput")
    y = nc.dram_tensor("y", [L, HID], BF16, kind="ExternalOutput")

    mm = nc.tensor.matmul

    with tile.TileContext(nc) as tc, ExitStack() as ctx:
        persist = ctx.enter_context(tc.tile_pool(name="persist", bufs=1))
        qn = [persist.tile([128, L], BF16, name=f"qn{h}", tag=f"qn{h}") for h in range(HPC)]
        qr = persist.tile([128, L], BF16, name="qr", tag="qr")
        kpe = [persist.tile([128, L], BF16, name=f"kpe{h}", tag=f"kpe{h}")
               for h in range(HPC)]
        latT = [persist.tile([128, L], BF16, name=f"latT{i}", tag=f"latT{i}") for i in range(NLAT)]
        kT = [persist.tile([128, L], BF16, name=f"kT{h}", tag=f"kT{h}") for h in range(HPC)]
        outT = [persist.tile([128, L], BF16, name=f"outT{h}", tag=f"outT{h}") for h in range(HPC)]
        vsb = persist.tile([128, NS * HPC * VDIM], BF16, name="vsb", tag="vsb")
        mtri = persist.tile([128, 128], BF16, name="mtri_sb", tag="mtri_sb")
        ones_col = persist.tile([128, 1], BF16, name="ones_col", tag="ones_col")
        ones_row = persist.tile([1, 128], BF16, name="ones_row", tag="ones_row")
        eps_col = persist.tile([1, 1], F32, name="eps_col", tag="eps_col")
        w_sb = [persist.tile([128, WCOLS], BF16, name=f"w{k}", tag=f"w{k}") for k in range(NK)]
        x_sb = [persist.tile([128, L], BF16, name=f"x{k}", tag=f"x{k}") for k in range(NK)]
        we_sb = [[persist.tile([128, NOPE], BF16, name=f"we{h}{i}", tag=f"we{h}{i}")
                  for i in range(NLAT)] for h in range(HPC)]
        wu_sb = [persist.tile([128, HPC * VDIM], BF16, name=f"wu{i}", tag=f"wu{i}")
                 for i in range(NLAT)]
        wo_sb = [persist.tile([128, HID], BF16, name=f"wo{hh}", tag=f"wo{hh}")
                 for hh in range(HPC)]
        rows = ctx.enter_context(tc.tile_pool(name="rows", bufs=2))
        sqp = ctx.enter_context(tc.tile_pool(name="sqp", bufs=1))

        nc.vector.memset(eps_col, EPS)
        nc.vector.memset(kpe[0][64:128, :], 0.0)
        nc.vector.memset(kpe[1][0:64, :], 0.0)

        # ---------------- P0: fused projections + pipelined rmsnorm ----------
        p0_stack = ExitStack()
        pp0 = p0_stack.enter_context(tc.tile_pool(name="pp0", bufs=1, space="PSUM"))

        def p0_mm(j):
            pss = [pp0.tile([128, LCH], F32, name=f"pm{m}", tag=f"pm{m}") for m in range(8)]
            for k in range(NK):
                if j == 0:
                    nc.sync.dma_start(out=w_sb[k], in_=wqkv[ts(k, 128), :])
                    nc.sync.dma_start(out=x_sb[k][:, 0 : 2 * LCH],
                                      in_=xT[ts(k, 128), 0 : 2 * LCH])
                if j == 1:
                    nc.sync.dma_start(out=x_sb[k][:, 2 * LCH : L],
                                      in_=xT[ts(k, 128), 2 * LCH : L])
                if j == 0 and k == 3:
                    nc.sync.dma_start(out=ones_col, in_=ones_col_d[:, :])
                    nc.sync.dma_start(out=ones_row, in_=ones_row_d[:, :])
                if j == 0 and k == 6:
                    nc.sync.dma_start(out=mtri, in_=mtri_d[:, :])
                xt = x_sb[k][:, ts(j, LCH)]
                for m, (c0, cw) in enumerate(MS):
                    mm(pss[m][:cw, :], (w_sb[k][:, ds(c0, cw)]), (xt),
                       start=(k == 0), stop=(k == NK - 1))
            return pss

        def p0_copy(j, pss):
            jc = ds(j * LCH, LCH)
            order = list(range(8)) if j == 0 else [7, 0, 1, 2, 3, 4, 5, 6]
            with nc.allow_low_precision(reason="bf16 activations"):
                for m in order:
                    if m == 7:
                        nc.vector.tensor_copy(kpe[0][0:64, jc], pss[7][0:64, :])
                        nc.sync.dma_start(out=kpe[1][64:128, jc],
                                          in_=kpe[0][0:64, jc])
                    elif m == 0:
                        nc.vector.tensor_copy(qn[0][:, jc], pss[0])
                    elif m == 1:
                        nc.vector.tensor_copy(qn[1][:, jc], pss[1])
                    elif m == 2:
                        nc.vector.tensor_copy(qr[:, jc], pss[2])
                    else:
                        nc.vector.tensor_copy(latT[m - 3][:, jc], pss[m])

        def p0_sqmul(j):
            jc = ds(j * LCH, LCH)
            sqs = [sqp.tile([128, LCH], BF16, name=f"sq{i}", tag=f"sq{i}")
                   for i in range(NLAT)]
            with nc.allow_low_precision(reason="bf16 squares"):
                for i in range(NLAT):
                    nc.vector.tensor_mul(sqs[i], latT[i][:, jc], latT[i][:, jc])
            return sqs

        def p0_ssq(j, sqs, pool, tag):
            ssq = pool.tile([1, LCH], F32, name="ssq", tag=tag)
            for i in range(NLAT):
                mm(ssq, (ones_col), (sqs[i]), start=(i == 0), stop=(i == NLAT - 1))
            ln_row = rows.tile([1, LCH], F32, name="ln_row", tag="lnrow")
            nc.scalar.activation(ln_row, ssq, AF.Ln, bias=eps_col[0:1, :],
                                 scale=1.0 / LORA)
            scale_row = rows.tile([1, LCH], BF16, name="scale_row", tag="scrow")
            with nc.allow_low_precision(reason="bf16 row for broadcast matmul"):
                nc.scalar.activation(scale_row, ln_row, AF.Exp, scale=-0.5)
            return scale_row

        def p0_norm(j, scale_row, pool, tag):
            jc = ds(j * LCH, LCH)
            bc = pool.tile([128, LCH], F32, name="bc", tag=tag)
            mm(bc, (ones_row), (scale_row), start=True, stop=True)
            with nc.allow_low_precision(reason="bf16 normalized latent"):
                for i in range(NLAT):
                    nc.vector.tensor_mul(latT[i][:, jc], latT[i][:, jc], bc)

        pss_h, sqs_h, row_h = {}, {}, {}
        for j in range(NJ):
            if j >= 2:
                p0_norm(j - 2, row_h[j - 2], pp0, "pm6")
            pss_h[j] = p0_mm(j)
            if j == 2:  # prefetch P2 weights behind burst-2's xt queue
                for h in range(HPC):
                    for i in range(NLAT):
                        nc.sync.dma_start(out=we_sb[h][i], in_=we[h, ts(i, 128), :])
                for i in range(NLAT):
                    nc.sync.dma_start(out=wu_sb[i], in_=wu[ts(i, 128), :])
            if j >= 1:
                sqs_h[j - 1] = p0_sqmul(j - 1)
            p0_copy(j, pss_h[j])
            if j >= 1:
                row_h[j - 1] = p0_ssq(j - 1, sqs_h[j - 1], pp0, "pm7")
        nc.sync.dma_start(out=wo_sb[0], in_=wo0[:, :])
        nc.sync.dma_start(out=wo_sb[1], in_=wo1[:, :])
        sqs_h[3] = p0_sqmul(3)
        p0_stack.close()

        # ---------------- P2: k/v embed, interleaved with rmsnorm finish -----
        p2_stack = ExitStack()
        pp2 = p2_stack.enter_context(tc.tile_pool(name="pp2", bufs=1, space="PSUM"))

        def p2_pv(si):
            pv = pp2.tile([128, HPC * VDIM], F32, name="pv", tag="pv", bufs=2)
            for i in range(NLAT):
                mm(pv, (latT[i][:, ts(si, 128)]), (wu_sb[i]),
                   start=(i == 0), stop=(i == NLAT - 1))
            with nc.allow_low_precision(reason="bf16 v"):
                nc.vector.tensor_copy(vsb[:, ds(si * HPC * VDIM, HPC * VDIM)], pv)

        def p2_kt(h, j):
            pk = pp2.tile([128, LCH], F32, name="pk", tag="pk", bufs=2)
            for i in range(NLAT):
                mm(pk, (we_sb[h][i]), (latT[i][:, ts(j, LCH)]),
                   start=(i == 0), stop=(i == NLAT - 1))
            with nc.allow_low_precision(reason="bf16 k"):
                nc.vector.tensor_copy(kT[h][:, ts(j, LCH)], pk)

        for si in range(8):
            p2_pv(si)
        row_h[3] = p0_ssq(3, sqs_h[3], pp2, "ssqx")
        p0_norm(2, row_h[2], pp2, "bcx")
        for si in range(8, 12):
            p2_pv(si)
        p2_kt(0, 0)
        p2_kt(1, 0)
        p0_norm(3, row_h[3], pp2, "bcx")
        for h in range(HPC):
            p2_kt(h, 1)
            p2_kt(h, 2)
        for si in range(12, 16):
            p2_pv(si)
        p2_kt(0, 3)
        p2_kt(1, 3)
        p2_stack.close()

        # ---------------- P3: causal attention (pipelined tails) + P4 --------
        with (
            tc.tile_pool(name="pp3", bufs=1, space="PSUM") as pp3,
            tc.tile_pool(name="epool", bufs=4) as epool,
            tc.tile_pool(name="ypool", bufs=2) as ypool,
            tc.tile_pool(name="bpool", bufs=2) as bpool,
        ):
            def attn_burst(j, h):
                nsi = 4 * j + 4
                jc0 = j * LCH
                pcs = pp3.tile([1, LCH], F32, name="pcs", tag="pcs", bufs=2)
                po = pp3.tile([128, LCH], F32, name="po", tag="po", bufs=2)
                pend = []

                def flush_one():
                    si2, c2, w2, e2 = pend.pop(0)
                    mm(pcs[:, ds(c2, w2)], (ones_col), (e2[:, ds(c2, w2)]),
                       start=(si2 == 0), stop=(si2 == nsi - 1))
                    mm(po[:, ds(c2, w2)],
                       (vsb[:, ds(si2 * HPC * VDIM + h * VDIM, VDIM)]),
                       (e2[:, ds(c2, w2)]),
                       start=(si2 == 0), stop=(si2 == nsi - 1))

                for si in range(nsi):
                    d = si - 4 * j
                    c0 = 128 * d if d >= 0 else 0
                    w = LCH - c0
                    ps = pp3.tile([128, LCH], F32, name="ps", tag="ps", bufs=3)
                    mm(ps[:, ds(c0, w)], (kT[h][:, ts(si, 128)]),
                       (qn[h][:, ds(jc0 + c0, w)]), start=True, stop=False)
                    mm(ps[:, ds(c0, w)], (kpe[h][:, ts(si, 128)]),
                       (qr[:, ds(jc0 + c0, w)]), start=False, stop=True)
                    e = epool.tile([128, LCH], BF16, name="e", tag="e")
                    with nc.allow_low_precision(reason="bf16 attn weights"):
                        nc.scalar.activation(e[:, ds(c0, w)], ps[:, ds(c0, w)],
                                             AF.Exp, scale=SCALE)
                        if d >= 0:
                            nc.vector.tensor_mul(e[:, ds(c0, 128)],
                                                 e[:, ds(c0, 128)], mtri)
                    pend.append((si, c0, w, e))
                    if len(pend) > 2:
                        flush_one()
                while pend:
                    flush_one()
                return pcs, po

            def attn_tail(j, h, pcs, po):
                lnr = rows.tile([1, LCH], F32, name="lnr", tag="lnr")
                nc.scalar.activation(lnr, pcs, AF.Ln)
                rrow = rows.tile([1, LCH], BF16, name="rrow", tag="rrow")
                with nc.allow_low_precision(reason="bf16 row for broadcast matmul"):
                    nc.scalar.activation(rrow, lnr, AF.Exp, scale=-1.0)
                pbc = pp3.tile([128, LCH], F32, name="pbc", tag="pbc", bufs=1)
                mm(pbc, (ones_row), (rrow), start=True, stop=True)
                bcs = bpool.tile([128, LCH], BF16, name="bcs", tag="bcs")
                with nc.allow_low_precision(reason="bf16 attn output"):
                    nc.vector.tensor_copy(bcs, pbc)
                    nc.vector.tensor_mul(outT[h][:, ts(j, LCH)], po, bcs)

            prev = None
            for j in (3, 2, 1, 0):  # longest bursts first: warm HAM early
                for h in range(HPC):
                    cur = (j, h) + attn_burst(j, h)
                    if prev is not None:
                        attn_tail(*prev)
                    prev = cur
            attn_tail(*prev)

            # ---- P4: partial o_proj y = outT.T @ Wo[2-head rows] ----
            for i in range(NS):
                ysb = ypool.tile([128, HID], BF16, name="ysb", tag="ysb")
                for n in range(NJ):
                    py = pp3.tile([128, LCH], F32, name="py", tag="ps", bufs=3)
                    mm(py, (outT[0][:, ts(i, 128)]), (wo_sb[0][:, ts(n, LCH)]),
                       start=True, stop=False)
                    mm(py, (outT[1][:, ts(i, 128)]), (wo_sb[1][:, ts(n, LCH)]),
                       start=False, stop=True)
                    with nc.allow_low_precision(reason="bf16 partial output"):
                        if n % 2 == 0:
                            nc.vector.tensor_copy(ysb[:, ts(n, LCH)], py)
                        else:
                            nc.scalar.copy(ysb[:, ts(n, LCH)], py)
                nc.sync.dma_start(out=y[ts(i, 128), :], in_=ysb)

    _split_excess_waits(nc)
    return nc


_NC_CACHE = None


def _get_nc():
    global _NC_CACHE
    if _NC_CACHE is None:
        _NC_CACHE = _build_nc()
    return _NC_CACHE


def _make_in_maps(x, Wq, Wkv_a, kv_ln_w, W_embed, W_unembed, Wo):
    xT = np.ascontiguousarray(
        np.asarray(x, dtype=np.float32)[0].T).astype(NPBF16)
    Wq = np.asarray(Wq, dtype=np.float32)
    Wkv_a = np.asarray(Wkv_a, dtype=np.float32)
    kv_ln_w = np.asarray(kv_ln_w, dtype=np.float32)
    W_embed = np.asarray(W_embed, dtype=np.float32)
    W_unembed = np.asarray(W_unembed, dtype=np.float32)
    Wo = np.asarray(Wo, dtype=np.float32)

    Wq3 = Wq.reshape(HID, H, QDIM)
    # triangular diagonal-band mask: mtri[p, c] = 1 iff c >= p
    idx = np.arange(128)
    mtri = (idx[None, :] >= idx[:, None]).astype(NPBF16)

    in_maps = []
    for c in range(NCORES):
        h0, h1 = HPC * c, HPC * c + 1
        wqkv = np.concatenate(
            [
                Wq3[:, h0, :NOPE],
                Wq3[:, h1, :NOPE],
                Wq3[:, h0, NOPE:],
                Wq3[:, h1, NOPE:],
                Wkv_a,
            ],
            axis=1,
        )
        we_ = np.ascontiguousarray(
            W_embed[[h0, h1]] * kv_ln_w[None, :, None]).astype(NPBF16)
        wu_ = np.ascontiguousarray(
            np.concatenate([W_unembed[h0].T, W_unembed[h1].T], axis=1)
            * kv_ln_w[:, None]).astype(NPBF16)
        in_maps.append(
            {
                "xT": xT,
                "wqkv": np.ascontiguousarray(wqkv).astype(NPBF16),
                "we": we_,
                "wu": wu_,
                "wo0": np.ascontiguousarray(
                    Wo[h0 * VDIM: (h0 + 1) * VDIM]).astype(NPBF16),
                "wo1": np.ascontiguousarray(
                    Wo[h1 * VDIM: (h1 + 1) * VDIM]).astype(NPBF16),
                "mtri": mtri,
                "ones_col_d": np.ones((128, 1), NPBF16),
                "ones_row_d": np.ones((1, 128), NPBF16),
            }
        )
    return in_maps


def run(trace=False, tmpdir=None, **inputs):
    """Run the SPMD kernel; returns (full_output, BassKernelResults)."""
    inputs.pop("mask", None)  # causal structure is hardcoded
    nc = _get_nc()
    in_maps = _make_in_maps(**inputs)
    res = run_bass_kernel_spmd(
        nc, in_maps, core_ids=list(range(NCORES)), trace=trace, tmpdir=tmpdir
    )
    y = np.zeros((L, HID), dtype=np.float32)
    for c in range(NCORES):
        y += np.asarray(res.results[c]["y"], dtype=np.float32)
    return y.reshape(B, L, HID), res


def kernel(**inputs):
    y, _ = run(trace=False, **inputs)
    return y


# revision 17
# speedup vs baseline: 1.0170x; 1.0003x over previous
"""Bass/Trainium2 kernel for Kimi-style MLA attention (nn_KimiMLAAttention).

Strategy (8 NeuronCores, tensor-parallel over heads):
  - 16 heads -> 2 heads per core. Each core computes q-projection for its 2
    heads, the (replicated) compressed-kv projection + rmsnorm, per-head
    k-embed / v-unembed from the shared latent, causal attention in a
    TRANSPOSED score layout (scores^T[s, l]), and a partial o_proj against
    its 2-head slice of Wo. Host sums the 8 partial outputs.

v2 performance notes (from the v1 trace):
  - All matmul operands are bf16 (PSUM accumulation stays fp32). Same PE
    rate as fp32r but half the DMA/SBUF traffic and shorter weight loads.
  - The PE clock is HAM-gated: any PE-idle gap re-throttles it to 1.2 GHz.
    v1 lost ~240us to oscillation because softmax/rmsnorm tails (DVE
    reciprocal -> broadcast matmul) sat in the PE FIFO. Here every slow
    tail is software-pipelined: its PE matmuls are emitted only after the
    NEXT compute burst, so the reciprocal runs concurrently with matmuls.
  - Causal diagonal s-tiles use trimmed moving dims (columns >= 128*d) and
    a single 128x128 triangular mask multiply.
"""

from contextlib import ExitStack

import numpy as np
import ml_dtypes

import concourse.bass as bass
import concourse.tile as tile
from concourse import mybir
from concourse.bass import ds, ts
from concourse.bass_utils import run_bass_kernel_spmd

F32 = mybir.dt.float32
F32R = mybir.dt.float32r
BF16 = mybir.dt.bfloat16
AF = mybir.ActivationFunctionType
NPBF16 = ml_dtypes.bfloat16


def _patch_tile_tail_drain():
    """walrus's CoreV3 codegen rejects the TileContext tail drain when it
    carries >1 sem waits ("Too many sync wait commands"). Split the waits
    across multiple single-wait drain instructions on the sync engine."""
    if getattr(tile.TileContext, "_tail_drain_patched", False):
        return
    from concourse.vector_clock import ScopedClock

    def _drain_and_barrier(self, tick_clock, wait_clock):
        nc = self.nc
        drain_inst = nc.sync.drain()
        wait_clock.add_sem_waits(
            drain_inst.ins, ScopedClock({None: tick_clock.global_clock})
        )
        inst = drain_inst.ins
        si = inst.sync_info
        if si is not None and si.on_wait is not None and len(si.on_wait) > 1:
            waits = list(si.on_wait)
            upd = list(si.on_update) if si.on_update else []
            inst.sync_info = mybir.SyncInfo(on_wait=waits[:1], on_update=[])
            for i, w in enumerate(waits[1:]):
                extra = nc.sync.drain()
                last = i == len(waits) - 2
                extra.ins.sync_info = mybir.SyncInfo(
                    on_wait=[w], on_update=upd if last else []
                )
        nc.all_engine_barrier()
        assert self.sems is not None
        popped = nc._tile_sem_poison_stack.pop()
        assert popped is self._sem_poison
        nc.clear_and_free_semaphores(list(self.sems.allocated().values()))
        nc.all_engine_barrier()

    tile.TileContext._drain_and_barrier = _drain_and_barrier
    tile.TileContext._tail_drain_patched = True


_patch_tile_tail_drain()


def _split_excess_waits(nc, max_waits=1):
    """walrus's per-instruction sync-wait slots are tiny on this compiler
    build; hoist excess sem waits onto same-engine NoOp carriers placed
    immediately before the instruction (waits fire earlier in the same
    engine stream, so ordering semantics are preserved)."""
    for f in nc.m.functions:
        for bb in f.blocks:
            insts = bb.instructions
            if not any(
                i.sync_info is not None
                and i.sync_info.on_wait
                and len(i.sync_info.on_wait) > max_waits
                for i in insts
            ):
                continue
            out = []
            for inst in insts:
                si = inst.sync_info
                if si is not None and si.on_wait and len(si.on_wait) > max_waits:
                    waits = list(si.on_wait)
                    for w in waits[:-max_waits]:
                        nop = mybir.InstNoOp(
                            name=nc.get_next_instruction_name(), ins=[], outs=[]
                        )
                        nop.engine = inst.engine
                        nop.sync_info = mybir.SyncInfo(on_wait=[w], on_update=[])
                        out.append(nop)
                    inst.sync_info = mybir.SyncInfo(
                        on_wait=waits[-max_waits:],
                        on_update=list(si.on_update) if si.on_update else [],
                    )
                out.append(inst)
            bb.instructions = out


B, L, HID = 1, 2048, 2048
H = 16
NOPE, ROPE, VDIM, LORA = 128, 64, 128, 512
QDIM = NOPE + ROPE
EPS = 1e-5
SCALE = QDIM**-0.5
NCORES = 8
HPC = H // NCORES  # 2 heads per core

LCH = 512  # moving-operand chunk (max moving free dim / PSUM bank)
NJ = L // LCH  # 4 l-chunks
NK = HID // 128  # 16 contraction tiles for projections
NS = L // 128  # 16 s(key)-tiles
NLAT = LORA // 128  # 4 latent partition tiles
WCOLS = 960  # fused projection weight columns
# m-chunks of wqkv columns: h0 nope, h1 nope, ropes, 4x latent, k_pe
MS = [(0, 128), (128, 128), (256, 128), (384, 128), (512, 128),
      (640, 128), (768, 128), (896, 64)]


def _build_nc():
    nc = bass.Bass()
    xT = nc.dram_tensor("xT", [HID, L], BF16, kind="ExternalInput")
    wqkv = nc.dram_tensor("wqkv", [HID, WCOLS], BF16, kind="ExternalInput")
    we = nc.dram_tensor("we", [HPC, LORA, NOPE], BF16, kind="ExternalInput")
    wu = nc.dram_tensor("wu", [LORA, HPC * VDIM], BF16, kind="ExternalInput")
    wo0 = nc.dram_tensor("wo0", [VDIM, HID], BF16, kind="ExternalInput")
    wo1 = nc.dram_tensor("wo1", [VDIM, HID], BF16, kind="ExternalInput")
    mtri_d = nc.dram_tensor("mtri", [128, 128], BF16, kind="ExternalInput")
    ones_col_d = nc.dram_tensor("ones_col_d", [128, 1], BF16, kind="ExternalInput")
    ones_row_d = nc.dram_tensor("ones_row_d", [1, 128], BF16, kind="ExternalInput")
    y = nc.dram_tensor("y", [L, HID], BF16, kind="ExternalOutput")

    mm = nc.tensor.matmul

    with tile.TileContext(nc) as tc, ExitStack() as ctx:
        persist = ctx.enter_context(tc.tile_pool(name="persist", bufs=1))
        qn = [persist.tile([128, L], BF16, name=f"qn{h}", tag=f"qn{h}") for h in range(HPC)]
        qr = persist.tile([128, L], BF16, name="qr", tag="qr")
        kpe = [persist.tile([128, L], BF16, name=f"kpe{h}", tag=f"kpe{h}")
               for h in range(HPC)]
        latT = [persist.tile([128, L], BF16, name=f"latT{i}", tag=f"latT{i}") for i in range(NLAT)]
        kT = [persist.tile([128, L], BF16, name=f"kT{h}", tag=f"kT{h}") for h in range(HPC)]
        outT = [persist.tile([128, L], BF16, name=f"outT{h}", tag=f"outT{h}") for h in range(HPC)]
        vsb = persist.tile([128, NS * HPC * VDIM], BF16, name="vsb", tag="vsb")
        mtri = persist.tile([128, 128], BF16, name="mtri_sb", tag="mtri_sb")
        ones_col = persist.tile([128, 1], BF16, name="ones_col", tag="ones_col")
        ones_row = persist.tile([1, 128], BF16, name="ones_row", tag="ones_row")
        eps_col = persist.tile([1, 1], F32, name="eps_col", tag="eps_col")
        w_sb = [persist.tile([128, WCOLS], BF16, name=f"w{k}", tag=f"w{k}") for k in range(NK)]
        x_sb = [persist.tile([128, L], BF16, name=f"x{k}", tag=f"x{k}") for k in range(NK)]
        we_sb = [[persist.tile([128, NOPE], BF16, name=f"we{h}{i}", tag=f"we{h}{i}")
                  for i in range(NLAT)] for h in range(HPC)]
        wu_sb = [persist.tile([128, HPC * VDIM], BF16, name=f"wu{i}", tag=f"wu{i}")
                 for i in range(NLAT)]
        wo_sb = [persist.tile([128, HID], BF16, name=f"wo{hh}", tag=f"wo{hh}")
                 for hh in range(HPC)]
        rows = ctx.enter_context(tc.tile_pool(name="rows", bufs=2))
        sqp = ctx.enter_context(tc.tile_pool(name="sqp", bufs=1))

        nc.vector.memset(eps_col, EPS)
        nc.vector.memset(kpe[0][64:128, :], 0.0)
        nc.vector.memset(kpe[1][0:64, :], 0.0)

        # ---------------- P0: fused projections + pipelined rmsnorm ----------
        p0_stack = ExitStack()
        pp0 = p0_stack.enter_context(tc.tile_pool(name="pp0", bufs=1, space="PSUM"))

        def p0_mm(j):
            pss = [pp0.tile([128, LCH], F32, name=f"pm{m}", tag=f"pm{m}") for m in range(8)]
            for k in range(NK):
                if j == 0:
                    nc.sync.dma_start(out=w_sb[k], in_=wqkv[ts(k, 128), :])
                    nc.sync.dma_start(out=x_sb[k][:, 0 : 2 * LCH],
                                      in_=xT[ts(k, 128), 0 : 2 * LCH])
                if j == 1:
                    nc.sync.dma_start(out=x_sb[k][:, 2 * LCH : L],
                                      in_=xT[ts(k, 128), 2 * LCH : L])
                if j == 0 and k == 3:
                    nc.sync.dma_start(out=ones_col, in_=ones_col_d[:, :])
                    nc.sync.dma_start(out=ones_row, in_=ones_row_d[:, :])
                if j == 0 and k == 6:
                    nc.sync.dma_start(out=mtri, in_=mtri_d[:, :])
                xt = x_sb[k][:, ts(j, LCH)]
                for m, (c0, cw) in enumerate(MS):
                    mm(pss[m][:cw, :], (w_sb[k][:, ds(c0, cw)]), (xt),
                       start=(k == 0), stop=(k == NK - 1))
            return pss

        def p0_copy(j, pss):
            jc = ds(j * LCH, LCH)
            order = list(range(8)) if j == 0 else [7, 0, 1, 2, 3, 4, 5, 6]
            with nc.allow_low_precision(reason="bf16 activations"):
                for m in order:
                    if m == 7:
                        nc.vector.tensor_copy(kpe[0][0:64, jc], pss[7][0:64, :])
                        nc.sync.dma_start(out=kpe[1][64:128, jc],
                                          in_=kpe[0][0:64, jc])
                    elif m == 0:
                        nc.vector.tensor_copy(qn[0][:, jc], pss[0])
                    elif m == 1:
                        nc.vector.tensor_copy(qn[1][:, jc], pss[1])
                    elif m == 2:
                        nc.vector.tensor_copy(qr[:, jc], pss[2])
                    else:
                        nc.vector.tensor_copy(latT[m - 3][:, jc], pss[m])

        def p0_sqmul(j):
            jc = ds(j * LCH, LCH)
            sqs = [sqp.tile([128, LCH], BF16, name=f"sq{i}", tag=f"sq{i}")
                   for i in range(NLAT)]
            with nc.allow_low_precision(reason="bf16 squares"):
                for i in range(NLAT):
                    nc.vector.tensor_mul(sqs[i], latT[i][:, jc], latT[i][:, jc])
            return sqs

        def p0_ssq(j, sqs, pool, tag):
            ssq = pool.tile([1, LCH], F32, name="ssq", tag=tag)
            for i in range(NLAT):
                mm(ssq, (ones_col), (sqs[i]), start=(i == 0), stop=(i == NLAT - 1))
            ln_row = rows.tile([1, LCH], F32, name="ln_row", tag="lnrow")
            nc.scalar.activation(ln_row, ssq, AF.Ln, bias=eps_col[0:1, :],
                                 scale=1.0 / LORA)
            scale_row = rows.tile([1, LCH], BF16, name="scale_row", tag="scrow")
            with nc.allow_low_precision(reason="bf16 row for broadcast matmul"):
                nc.scalar.activation(scale_row, ln_row, AF.Exp, scale=-0.5)
            return scale_row

        def p0_norm(j, scale_row, pool, tag):
            jc = ds(j * LCH, LCH)
            bc = pool.tile([128, LCH], F32, name="bc", tag=tag)
            mm(bc, (ones_row), (scale_row), start=True, stop=True)
            with nc.allow_low_precision(reason="bf16 normalized latent"):
                for i in range(NLAT):
                    nc.vector.tensor_mul(latT[i][:, jc], latT[i][:, jc], bc)

        pss_h, sqs_h, row_h = {}, {}, {}
        for j in range(NJ):
            if j >= 2:
                p0_norm(j - 2, row_h[j - 2], pp0, "pm6")
            pss_h[j] = p0_mm(j)
            if j == 2:  # prefetch P2 weights behind burst-2's xt queue
                for h in range(HPC):
                    for i in range(NLAT):
                        nc.sync.dma_start(out=we_sb[h][i], in_=we[h, ts(i, 128), :])
                for i in range(NLAT):
                    nc.sync.dma_start(out=wu_sb[i], in_=wu[ts(i, 128), :])
            if j >= 1:
                sqs_h[j - 1] = p0_sqmul(j - 1)
            p0_copy(j, pss_h[j])
            if j >= 1:
                row_h[j - 1] = p0_ssq(j - 1, sqs_h[j - 1], pp0, "pm7")
        nc.sync.dma_start(out=wo_sb[0], in_=wo0[:, :])
        nc.sync.dma_start(out=wo_sb[1], in_=wo1[:, :])
        sqs_h[3] = p0_sqmul(3)
        p0_stack.close()

        # ---------------- P2: k/v embed, interleaved with rmsnorm finish -----
        p2_stack = ExitStack()
        pp2 = p2_stack.enter_context(tc.tile_pool(name="pp2", bufs=1, space="PSUM"))

        def p2_pv(si):
            pv = pp2.tile([128, HPC * VDIM], F32, name="pv", tag="pv", bufs=2)
            for i in range(NLAT):
                mm(pv, (latT[i][:, ts(si, 128)]), (wu_sb[i]),
                   start=(i == 0), stop=(i == NLAT - 1))
            with nc.allow_low_precision(reason="bf16 v"):
                nc.vector.tensor_copy(vsb[:, ds(si * HPC * VDIM, HPC * VDIM)], pv)

        def p2_kt(h, j):
            pk = pp2.tile([128, LCH], F32, name="pk", tag="pk", bufs=2)
            for i in range(NLAT):
                mm(pk, (we_sb[h][i]), (latT[i][:, ts(j, LCH)]),
                   start=(i == 0), stop=(i == NLAT - 1))
            with nc.allow_low_precision(reason="bf16 k"):
                nc.vector.tensor_copy(kT[h][:, ts(j, LCH)], pk)

        for si in range(8):
            p2_pv(si)
        row_h[3] = p0_ssq(3, sqs_h[3], pp2, "ssqx")
        p0_norm(2, row_h[2], pp2, "bcx")
        for si in range(8, 12):
            p2_pv(si)
        p2_kt(0, 0)
        p2_kt(1, 0)
        p0_norm(3, row_h[3], pp2, "bcx")
        for h in range(HPC):
            p2_kt(h, 1)
            p2_kt(h, 2)
        for si in range(12, 16):
            p2_pv(si)
        p2_kt(0, 3)
        p2_kt(1, 3)
        p2_stack.close()

        # ---------------- P3: causal attention (pipelined tails) + P4 --------
        with (
            tc.tile_pool(name="pp3", bufs=1, space="PSUM") as pp3,
            tc.tile_pool(name="epool", bufs=4) as epool,
            tc.tile_pool(name="ypool", bufs=2) as ypool,
            tc.tile_pool(name="bpool", bufs=2) as bpool,
        ):
            def attn_burst(j, h):
                nsi = 4 * j + 4
                jc0 = j * LCH
                pcs = pp3.tile([1, LCH], F32, name="pcs", tag="pcs", bufs=1)
                po = pp3.tile([128, LCH], F32, name="po", tag="po", bufs=3)
                pend = []

                def flush_one():
                    si2, c2, w2, e2 = pend.pop(0)
                    mm(pcs[:, ds(c2, w2)], (ones_col), (e2[:, ds(c2, w2)]),
                       start=(si2 == 0), stop=(si2 == nsi - 1))
                    mm(po[:, ds(c2, w2)],
                       (vsb[:, ds(si2 * HPC * VDIM + h * VDIM, VDIM)]),
                       (e2[:, ds(c2, w2)]),
                       start=(si2 == 0), stop=(si2 == nsi - 1))

                for si in range(nsi):
                    d = si - 4 * j
                    c0 = 128 * d if d >= 0 else 0
                    w = LCH - c0
                    ps = pp3.tile([128, LCH], F32, name="ps", tag="ps", bufs=3)
                    mm(ps[:, ds(c0, w)], (kT[h][:, ts(si, 128)]),
                       (qn[h][:, ds(jc0 + c0, w)]), start=True, stop=False)
                    mm(ps[:, ds(c0, w)], (kpe[h][:, ts(si, 128)]),
                       (qr[:, ds(jc0 + c0, w)]), start=False, stop=True)
                    e = epool.tile([128, LCH], BF16, name="e", tag="e")
                    with nc.allow_low_precision(reason="bf16 attn weights"):
                        nc.scalar.activation(e[:, ds(c0, w)], ps[:, ds(c0, w)],
                                             AF.Exp, scale=SCALE)
                        if d >= 0:
                            nc.vector.tensor_mul(e[:, ds(c0, 128)],
                                                 e[:, ds(c0, 128)], mtri)
                    pend.append((si, c0, w, e))
                    if len(pend) > 2:
                        flush_one()
                while pend:
                    flush_one()
                return pcs, po

            def attn_tail(j, h, pcs, po):
                lnr = rows.tile([1, LCH], F32, name="lnr", tag="lnr")
                nc.scalar.activation(lnr, pcs, AF.Ln)
                rrow = rows.tile([1, LCH], BF16, name="rrow", tag="rrow")
                with nc.allow_low_precision(reason="bf16 row for broadcast matmul"):
                    nc.scalar.activation(rrow, lnr, AF.Exp, scale=-1.0)
                pbc = pp3.tile([128, LCH], F32, name="pbc", tag="pbc", bufs=1)
                mm(pbc, (ones_row), (rrow), start=True, stop=True)
                bcs = bpool.tile([128, LCH], BF16, name="bcs", tag="bcs")
                with nc.allow_low_precision(reason="bf16 attn output"):
                    nc.vector.tensor_copy(bcs, pbc)
                    nc.vector.tensor_mul(outT[h][:, ts(j, LCH)], po, bcs)

            prev = None
            for j in (3, 2, 1, 0):  # longest bursts first: warm HAM early
                for h in range(HPC):
                    cur = (j, h) + attn_burst(j, h)
                    if prev is not None:
                        attn_tail(*prev)
                    prev = cur
            attn_tail(*prev)

            # ---- P4: partial o_proj y = outT.T @ Wo[2-head rows] ----
            for i in range(NS):
                ysb = ypool.tile([128, HID], BF16, name="ysb", tag="ysb")
                for n in range(NJ):
                    py = pp3.tile([128, LCH], F32, name="py", tag="ps", bufs=3)
                    mm(py, (outT[0][:, ts(i, 128)]), (wo_sb[0][:, ts(n, LCH)]),
                       start=True, stop=False)
                    mm(py, (outT[1][:, ts(i, 128)]), (wo_sb[1][:, ts(n, LCH)]),
                       start=False, stop=True)
                    with nc.allow_low_precision(reason="bf16 partial output"):
                        if n % 2 == 0:
                            nc.vector.tensor_copy(ysb[:, ts(n, LCH)], py)
                        else:
                            nc.scalar.copy(ysb[:, ts(n, LCH)], py)
                nc.sync.dma_start(out=y[ts(i, 128), :], in_=ysb)

    _split_excess_waits(nc)
    return nc


_NC_CACHE = None


def _get_nc():
    global _NC_CACHE
    if _NC_CACHE is None:
        _NC_CACHE = _build_nc()
    return _NC_CACHE


def _make_in_maps(x, Wq, Wkv_a, kv_ln_w, W_embed, W_unembed, Wo):
    xT = np.ascontiguousarray(
        np.asarray(x, dtype=np.float32)[0].T).astype(NPBF16)
    Wq = np.asarray(Wq, dtype=np.float32)
    Wkv_a = np.asarray(Wkv_a, dtype=np.float32)
    kv_ln_w = np.asarray(kv_ln_w, dtype=np.float32)
    W_embed = np.asarray(W_embed, dtype=np.float32)
    W_unembed = np.asarray(W_unembed, dtype=np.float32)
    Wo = np.asarray(Wo, dtype=np.float32)

    Wq3 = Wq.reshape(HID, H, QDIM)
    # triangular diagonal-band mask: mtri[p, c] = 1 iff c >= p
    idx = np.arange(128)
    mtri = (idx[None, :] >= idx[:, None]).astype(NPBF16)

    in_maps = []
    for c in range(NCORES):
        h0, h1 = HPC * c, HPC * c + 1
        wqkv = np.concatenate(
            [
                Wq3[:, h0, :NOPE],
                Wq3[:, h1, :NOPE],
                Wq3[:, h0, NOPE:],
                Wq3[:, h1, NOPE:],
                Wkv_a,
            ],
            axis=1,
        )
        we_ = np.ascontiguousarray(
            W_embed[[h0, h1]] * kv_ln_w[None, :, None]).astype(NPBF16)
        wu_ = np.ascontiguousarray(
            np.concatenate([W_unembed[h0].T, W_unembed[h1].T], axis=1)
            * kv_ln_w[:, None]).astype(NPBF16)
        in_maps.append(
            {
                "xT": xT,
                "wqkv": np.ascontiguousarray(wqkv).astype(NPBF16),
                "we": we_,
                "wu": wu_,
                "wo0": np.ascontiguousarray(
                    Wo[h0 * VDIM: (h0 + 1) * VDIM]).astype(NPBF16),
                "wo1": np.ascontiguousarray(
                    Wo[h1 * VDIM: (h1 + 1) * VDIM]).astype(NPBF16),
                "mtri": mtri,
                "ones_col_d": np.ones((128, 1), NPBF16),
                "ones_row_d": np.ones((1, 128), NPBF16),
            }
        )
    return in_maps


def run(trace=False, tmpdir=None, **inputs):
    """Run the SPMD kernel; returns (full_output, BassKernelResults)."""
    inputs.pop("mask", None)  # causal structure is hardcoded
    nc = _get_nc()
    in_maps = _make_in_maps(**inputs)
    res = run_bass_kernel_spmd(
        nc, in_maps, core_ids=list(range(NCORES)), trace=trace, tmpdir=tmpdir
    )
    y = np.zeros((L, HID), dtype=np.float32)
    for c in range(NCORES):
        y += np.asarray(res.results[c]["y"], dtype=np.float32)
    return y.reshape(B, L, HID), res


def kernel(**inputs):
    y, _ = run(trace=False, **inputs)
    return y


# revision 18
# speedup vs baseline: 1.0236x; 1.0065x over previous
"""Bass/Trainium2 kernel for Kimi-style MLA attention (nn_KimiMLAAttention).

Strategy (8 NeuronCores, tensor-parallel over heads):
  - 16 heads -> 2 heads per core. Each core computes q-projection for its 2
    heads, the (replicated) compressed-kv projection + rmsnorm, per-head
    k-embed / v-unembed from the shared latent, causal attention in a
    TRANSPOSED score layout (scores^T[s, l]), and a partial o_proj against
    its 2-head slice of Wo. Host sums the 8 partial outputs.

v2 performance notes (from the v1 trace):
  - All matmul operands are bf16 (PSUM accumulation stays fp32). Same PE
    rate as fp32r but half the DMA/SBUF traffic and shorter weight loads.
  - The PE clock is HAM-gated: any PE-idle gap re-throttles it to 1.2 GHz.
    v1 lost ~240us to oscillation because softmax/rmsnorm tails (DVE
    reciprocal -> broadcast matmul) sat in the PE FIFO. Here every slow
    tail is software-pipelined: its PE matmuls are emitted only after the
    NEXT compute burst, so the reciprocal runs concurrently with matmuls.
  - Causal diagonal s-tiles use trimmed moving dims (columns >= 128*d) and
    a single 128x128 triangular mask multiply.
"""

from contextlib import ExitStack

import numpy as np
import ml_dtypes

import concourse.bass as bass
import concourse.tile as tile
from concourse import mybir
from concourse.bass import ds, ts
from concourse.bass_utils import run_bass_kernel_spmd

F32 = mybir.dt.float32
F32R = mybir.dt.float32r
BF16 = mybir.dt.bfloat16
AF = mybir.ActivationFunctionType
NPBF16 = ml_dtypes.bfloat16


def _patch_tile_tail_drain():
    """walrus's CoreV3 codegen rejects the TileContext tail drain when it
    carries >1 sem waits ("Too many sync wait commands"). Split the waits
    across multiple single-wait drain instructions on the sync engine."""
    if getattr(tile.TileContext, "_tail_drain_patched", False):
        return
    from concourse.vector_clock import ScopedClock

    def _drain_and_barrier(self, tick_clock, wait_clock):
        nc = self.nc
        drain_inst = nc.sync.drain()
        wait_clock.add_sem_waits(
            drain_inst.ins, ScopedClock({None: tick_clock.global_clock})
        )
        inst = drain_inst.ins
        si = inst.sync_info
        if si is not None and si.on_wait is not None and len(si.on_wait) > 1:
            waits = list(si.on_wait)
            upd = list(si.on_update) if si.on_update else []
            inst.sync_info = mybir.SyncInfo(on_wait=waits[:1], on_update=[])
            for i, w in enumerate(waits[1:]):
                extra = nc.sync.drain()
                last = i == len(waits) - 2
                extra.ins.sync_info = mybir.SyncInfo(
                    on_wait=[w], on_update=upd if last else []
                )
        nc.all_engine_barrier()
        assert self.sems is not None
        popped = nc._tile_sem_poison_stack.pop()
        assert popped is self._sem_poison
        nc.clear_and_free_semaphores(list(self.sems.allocated().values()))
        nc.all_engine_barrier()

    tile.TileContext._drain_and_barrier = _drain_and_barrier
    tile.TileContext._tail_drain_patched = True


_patch_tile_tail_drain()


def _split_excess_waits(nc, max_waits=1):
    """walrus's per-instruction sync-wait slots are tiny on this compiler
    build; hoist excess sem waits onto same-engine NoOp carriers placed
    immediately before the instruction (waits fire earlier in the same
    engine stream, so ordering semantics are preserved)."""
    for f in nc.m.functions:
        for bb in f.blocks:
            insts = bb.instructions
            if not any(
                i.sync_info is not None
                and i.sync_info.on_wait
                and len(i.sync_info.on_wait) > max_waits
                for i in insts
            ):
                continue
            out = []
            for inst in insts:
                si = inst.sync_info
                if si is not None and si.on_wait and len(si.on_wait) > max_waits:
                    waits = list(si.on_wait)
                    for w in waits[:-max_waits]:
                        nop = mybir.InstNoOp(
                            name=nc.get_next_instruction_name(), ins=[], outs=[]
                        )
                        nop.engine = inst.engine
                        nop.sync_info = mybir.SyncInfo(on_wait=[w], on_update=[])
                        out.append(nop)
                    inst.sync_info = mybir.SyncInfo(
                        on_wait=waits[-max_waits:],
                        on_update=list(si.on_update) if si.on_update else [],
                    )
                out.append(inst)
            bb.instructions = out


B, L, HID = 1, 2048, 2048
H = 16
NOPE, ROPE, VDIM, LORA = 128, 64, 128, 512
QDIM = NOPE + ROPE
EPS = 1e-5
SCALE = QDIM**-0.5
NCORES = 8
HPC = H // NCORES  # 2 heads per core

LCH = 512  # moving-operand chunk (max moving free dim / PSUM bank)
NJ = L // LCH  # 4 l-chunks
NK = HID // 128  # 16 contraction tiles for projections
NS = L // 128  # 16 s(key)-tiles
NLAT = LORA // 128  # 4 latent partition tiles
WCOLS = 960  # fused projection weight columns
# m-chunks of wqkv columns: h0 nope, h1 nope, ropes, 4x latent, k_pe
MS = [(0, 128), (128, 128), (256, 128), (384, 128), (512, 128),
      (640, 128), (768, 128), (896, 64)]


def _build_nc():
    nc = bass.Bass()
    xT = nc.dram_tensor("xT", [HID, L], BF16, kind="ExternalInput")
    wqkv = nc.dram_tensor("wqkv", [HID, WCOLS], BF16, kind="ExternalInput")
    we = nc.dram_tensor("we", [HPC, LORA, NOPE], BF16, kind="ExternalInput")
    wu = nc.dram_tensor("wu", [LORA, HPC * VDIM], BF16, kind="ExternalInput")
    wo0 = nc.dram_tensor("wo0", [VDIM, HID], BF16, kind="ExternalInput")
    wo1 = nc.dram_tensor("wo1", [VDIM, HID], BF16, kind="ExternalInput")
    mtri_d = nc.dram_tensor("mtri", [128, 128], BF16, kind="ExternalInput")
    ones_col_d = nc.dram_tensor("ones_col_d", [128, 1], BF16, kind="ExternalInput")
    ones_row_d = nc.dram_tensor("ones_row_d", [1, 128], BF16, kind="ExternalInput")
    y = nc.dram_tensor("y", [L, HID], BF16, kind="ExternalOutput")

    mm = nc.tensor.matmul

    with tile.TileContext(nc) as tc, ExitStack() as ctx:
        persist = ctx.enter_context(tc.tile_pool(name="persist", bufs=1))
        qn = [persist.tile([128, L], BF16, name=f"qn{h}", tag=f"qn{h}") for h in range(HPC)]
        qr = persist.tile([128, L], BF16, name="qr", tag="qr")
        kpe = [persist.tile([128, L], BF16, name=f"kpe{h}", tag=f"kpe{h}")
               for h in range(HPC)]
        latT = [persist.tile([128, L], BF16, name=f"latT{i}", tag=f"latT{i}") for i in range(NLAT)]
        kT = [persist.tile([128, L], BF16, name=f"kT{h}", tag=f"kT{h}") for h in range(HPC)]
        outT = [persist.tile([128, L], BF16, name=f"outT{h}", tag=f"outT{h}") for h in range(HPC)]
        vsb = persist.tile([128, NS * HPC * VDIM], BF16, name="vsb", tag="vsb")
        mtri = persist.tile([128, 128], BF16, name="mtri_sb", tag="mtri_sb")
        ones_col = persist.tile([128, 1], BF16, name="ones_col", tag="ones_col")
        ones_row = persist.tile([1, 128], BF16, name="ones_row", tag="ones_row")
        eps_col = persist.tile([1, 1], F32, name="eps_col", tag="eps_col")
        w_sb = [persist.tile([128, WCOLS], BF16, name=f"w{k}", tag=f"w{k}") for k in range(NK)]
        x_sb = [persist.tile([128, L], BF16, name=f"x{k}", tag=f"x{k}") for k in range(NK)]
        we_sb = [[persist.tile([128, NOPE], BF16, name=f"we{h}{i}", tag=f"we{h}{i}")
                  for i in range(NLAT)] for h in range(HPC)]
        wu_sb = [persist.tile([128, HPC * VDIM], BF16, name=f"wu{i}", tag=f"wu{i}")
                 for i in range(NLAT)]
        wo_sb = [persist.tile([128, HID], BF16, name=f"wo{hh}", tag=f"wo{hh}")
                 for hh in range(HPC)]
        rows = ctx.enter_context(tc.tile_pool(name="rows", bufs=2))
        sqp = ctx.enter_context(tc.tile_pool(name="sqp", bufs=1))

        nc.vector.memset(eps_col, EPS)
        nc.vector.memset(kpe[0][64:128, :], 0.0)
        nc.vector.memset(kpe[1][0:64, :], 0.0)

        # ---------------- P0: fused projections + pipelined rmsnorm ----------
        p0_stack = ExitStack()
        pp0 = p0_stack.enter_context(tc.tile_pool(name="pp0", bufs=1, space="PSUM"))

        def p0_mm(j):
            pss = [pp0.tile([128, LCH], F32, name=f"pm{m}", tag=f"pm{m}") for m in range(8)]
            for k in range(NK):
                if j == 0:
                    nc.sync.dma_start(out=w_sb[k], in_=wqkv[ts(k, 128), :])
                    nc.sync.dma_start(out=x_sb[k][:, 0 : 2 * LCH],
                                      in_=xT[ts(k, 128), 0 : 2 * LCH])
                if j == 1:
                    nc.sync.dma_start(out=x_sb[k][:, 2 * LCH : L],
                                      in_=xT[ts(k, 128), 2 * LCH : L])
                if j == 0 and k == 3:
                    nc.sync.dma_start(out=ones_col, in_=ones_col_d[:, :])
                    nc.sync.dma_start(out=ones_row, in_=ones_row_d[:, :])
                if j == 0 and k == 6:
                    nc.sync.dma_start(out=mtri, in_=mtri_d[:, :])
                xt = x_sb[k][:, ts(j, LCH)]
                morder = range(8) if (j == 0 or k > 0) else [7, 0, 1, 2, 3, 4, 5, 6]
                for m in morder:
                    c0, cw = MS[m]
                    mm(pss[m][:cw, :], (w_sb[k][:, ds(c0, cw)]), (xt),
                       start=(k == 0), stop=(k == NK - 1))
            return pss

        def p0_copy(j, pss):
            jc = ds(j * LCH, LCH)
            order = list(range(8)) if j == 0 else [7, 0, 1, 2, 3, 4, 5, 6]
            dests = {0: qn[0][:, jc], 1: qn[1][:, jc], 2: qr[:, jc],
                     3: latT[0][:, jc], 4: latT[1][:, jc],
                     5: latT[2][:, jc], 6: latT[3][:, jc]}
            with nc.allow_low_precision(reason="bf16 activations"):
                for idx, m in enumerate(order):
                    if m == 7:
                        nc.vector.tensor_copy(kpe[0][0:64, jc], pss[7][0:64, :])
                        nc.sync.dma_start(out=kpe[1][64:128, jc],
                                          in_=kpe[0][0:64, jc])
                    elif idx % 2 == 0:
                        nc.vector.tensor_copy(dests[m], pss[m])
                    else:
                        nc.scalar.copy(dests[m], pss[m])

        def p0_sqmul(j):
            jc = ds(j * LCH, LCH)
            sqs = [sqp.tile([128, LCH], BF16, name=f"sq{i}", tag=f"sq{i}")
                   for i in range(NLAT)]
            with nc.allow_low_precision(reason="bf16 squares"):
                for i in range(NLAT):
                    nc.vector.tensor_mul(sqs[i], latT[i][:, jc], latT[i][:, jc])
            return sqs

        def p0_ssq(j, sqs, pool, tag):
            ssq = pool.tile([1, LCH], F32, name="ssq", tag=tag)
            for i in range(NLAT):
                mm(ssq, (ones_col), (sqs[i]), start=(i == 0), stop=(i == NLAT - 1))
            ln_row = rows.tile([1, LCH], F32, name="ln_row", tag="lnrow")
            nc.scalar.activation(ln_row, ssq, AF.Ln, bias=eps_col[0:1, :],
                                 scale=1.0 / LORA)
            scale_row = rows.tile([1, LCH], BF16, name="scale_row", tag="scrow")
            with nc.allow_low_precision(reason="bf16 row for broadcast matmul"):
                nc.scalar.activation(scale_row, ln_row, AF.Exp, scale=-0.5)
            return scale_row

        def p0_norm(j, scale_row, pool, tag):
            jc = ds(j * LCH, LCH)
            bc = pool.tile([128, LCH], F32, name="bc", tag=tag)
            mm(bc, (ones_row), (scale_row), start=True, stop=True)
            with nc.allow_low_precision(reason="bf16 normalized latent"):
                for i in range(NLAT):
                    nc.vector.tensor_mul(latT[i][:, jc], latT[i][:, jc], bc)

        pss_h, sqs_h, row_h = {}, {}, {}
        for j in range(NJ):
            if j >= 2:
                p0_norm(j - 2, row_h[j - 2], pp0, "pm6")
            pss_h[j] = p0_mm(j)
            if j == 2:  # prefetch P2 weights behind burst-2's xt queue
                for h in range(HPC):
                    for i in range(NLAT):
                        nc.sync.dma_start(out=we_sb[h][i], in_=we[h, ts(i, 128), :])
                for i in range(NLAT):
                    nc.sync.dma_start(out=wu_sb[i], in_=wu[ts(i, 128), :])
            if j >= 1:
                sqs_h[j - 1] = p0_sqmul(j - 1)
            p0_copy(j, pss_h[j])
            if j >= 1:
                row_h[j - 1] = p0_ssq(j - 1, sqs_h[j - 1], pp0, "pm7")
        nc.sync.dma_start(out=wo_sb[0], in_=wo0[:, :])
        nc.sync.dma_start(out=wo_sb[1], in_=wo1[:, :])
        sqs_h[3] = p0_sqmul(3)
        p0_stack.close()

        # ---------------- P2: k/v embed, interleaved with rmsnorm finish -----
        p2_stack = ExitStack()
        pp2 = p2_stack.enter_context(tc.tile_pool(name="pp2", bufs=1, space="PSUM"))

        def p2_pv(si):
            pv = pp2.tile([128, HPC * VDIM], F32, name="pv", tag="pv", bufs=2)
            for i in range(NLAT):
                mm(pv, (latT[i][:, ts(si, 128)]), (wu_sb[i]),
                   start=(i == 0), stop=(i == NLAT - 1))
            with nc.allow_low_precision(reason="bf16 v"):
                nc.vector.tensor_copy(vsb[:, ds(si * HPC * VDIM, HPC * VDIM)], pv)

        def p2_kt(h, j):
            pk = pp2.tile([128, LCH], F32, name="pk", tag="pk", bufs=2)
            for i in range(NLAT):
                mm(pk, (we_sb[h][i]), (latT[i][:, ts(j, LCH)]),
                   start=(i == 0), stop=(i == NLAT - 1))
            with nc.allow_low_precision(reason="bf16 k"):
                nc.vector.tensor_copy(kT[h][:, ts(j, LCH)], pk)

        for si in range(8):
            p2_pv(si)
        row_h[3] = p0_ssq(3, sqs_h[3], pp2, "ssqx")
        p0_norm(2, row_h[2], pp2, "bcx")
        for si in range(8, 12):
            p2_pv(si)
        p2_kt(0, 0)
        p2_kt(1, 0)
        p0_norm(3, row_h[3], pp2, "bcx")
        for h in range(HPC):
            p2_kt(h, 1)
            p2_kt(h, 2)
        for si in range(12, 16):
            p2_pv(si)
        p2_kt(0, 3)
        p2_kt(1, 3)
        p2_stack.close()

        # ---------------- P3: causal attention (pipelined tails) + P4 --------
        with (
            tc.tile_pool(name="pp3", bufs=1, space="PSUM") as pp3,
            tc.tile_pool(name="epool", bufs=4) as epool,
            tc.tile_pool(name="ypool", bufs=2) as ypool,
            tc.tile_pool(name="bpool", bufs=2) as bpool,
        ):
            def attn_burst(j, h):
                nsi = 4 * j + 4
                jc0 = j * LCH
                pcs = pp3.tile([1, LCH], F32, name="pcs", tag="pcs", bufs=1)
                po = pp3.tile([128, LCH], F32, name="po", tag="po", bufs=3)
                pend = []

                def flush_one():
                    si2, c2, w2, e2 = pend.pop(0)
                    mm(pcs[:, ds(c2, w2)], (ones_col), (e2[:, ds(c2, w2)]),
                       start=(si2 == 0), stop=(si2 == nsi - 1))
                    mm(po[:, ds(c2, w2)],
                       (vsb[:, ds(si2 * HPC * VDIM + h * VDIM, VDIM)]),
                       (e2[:, ds(c2, w2)]),
                       start=(si2 == 0), stop=(si2 == nsi - 1))

                for si in range(nsi):
                    d = si - 4 * j
                    c0 = 128 * d if d >= 0 else 0
                    w = LCH - c0
                    ps = pp3.tile([128, LCH], F32, name="ps", tag="ps", bufs=3)
                    mm(ps[:, ds(c0, w)], (kT[h][:, ts(si, 128)]),
                       (qn[h][:, ds(jc0 + c0, w)]), start=True, stop=False)
                    mm(ps[:, ds(c0, w)], (kpe[h][:, ts(si, 128)]),
                       (qr[:, ds(jc0 + c0, w)]), start=False, stop=True)
                    e = epool.tile([128, LCH], BF16, name="e", tag="e")
                    with nc.allow_low_precision(reason="bf16 attn weights"):
                        nc.scalar.activation(e[:, ds(c0, w)], ps[:, ds(c0, w)],
                                             AF.Exp, scale=SCALE)
                        if d >= 0:
                            nc.vector.tensor_mul(e[:, ds(c0, 128)],
                                                 e[:, ds(c0, 128)], mtri)
                    pend.append((si, c0, w, e))
                    if len(pend) > 2:
                        flush_one()
                while pend:
                    flush_one()
                return pcs, po

            def attn_tail(j, h, pcs, po):
                lnr = rows.tile([1, LCH], F32, name="lnr", tag="lnr")
                nc.scalar.activation(lnr, pcs, AF.Ln)
                rrow = rows.tile([1, LCH], BF16, name="rrow", tag="rrow")
                with nc.allow_low_precision(reason="bf16 row for broadcast matmul"):
                    nc.scalar.activation(rrow, lnr, AF.Exp, scale=-1.0)
                pbc = pp3.tile([128, LCH], F32, name="pbc", tag="pbc", bufs=1)
                mm(pbc, (ones_row), (rrow), start=True, stop=True)
                bcs = bpool.tile([128, LCH], BF16, name="bcs", tag="bcs")
                with nc.allow_low_precision(reason="bf16 attn output"):
                    nc.vector.tensor_copy(bcs, pbc)
                    nc.vector.tensor_mul(outT[h][:, ts(j, LCH)], po, bcs)

            prev = None
            for j in (3, 2, 1, 0):  # longest bursts first: warm HAM early
                for h in range(HPC):
                    cur = (j, h) + attn_burst(j, h)
                    if prev is not None:
                        attn_tail(*prev)
                    prev = cur
            attn_tail(*prev)

            # ---- P4: partial o_proj y = outT.T @ Wo[2-head rows] ----
            for i in range(NS):
                ysb = ypool.tile([128, HID], BF16, name="ysb", tag="ysb")
                for n in range(NJ):
                    py = pp3.tile([128, LCH], F32, name="py", tag="ps", bufs=3)
                    mm(py, (outT[0][:, ts(i, 128)]), (wo_sb[0][:, ts(n, LCH)]),
                       start=True, stop=False)
                    mm(py, (outT[1][:, ts(i, 128)]), (wo_sb[1][:, ts(n, LCH)]),
                       start=False, stop=True)
                    with nc.allow_low_precision(reason="bf16 partial output"):
                        if n % 2 == 0:
                            nc.vector.tensor_copy(ysb[:, ts(n, LCH)], py)
                        else:
                            nc.scalar.copy(ysb[:, ts(n, LCH)], py)
                nc.sync.dma_start(out=y[ts(i, 128), :], in_=ysb)

    _split_excess_waits(nc)
    return nc


_NC_CACHE = None


def _get_nc():
    global _NC_CACHE
    if _NC_CACHE is None:
        _NC_CACHE = _build_nc()
    return _NC_CACHE


def _make_in_maps(x, Wq, Wkv_a, kv_ln_w, W_embed, W_unembed, Wo):
    xT = np.ascontiguousarray(
        np.asarray(x, dtype=np.float32)[0].T).astype(NPBF16)
    Wq = np.asarray(Wq, dtype=np.float32)
    Wkv_a = np.asarray(Wkv_a, dtype=np.float32)
    kv_ln_w = np.asarray(kv_ln_w, dtype=np.float32)
    W_embed = np.asarray(W_embed, dtype=np.float32)
    W_unembed = np.asarray(W_unembed, dtype=np.float32)
    Wo = np.asarray(Wo, dtype=np.float32)

    Wq3 = Wq.reshape(HID, H, QDIM)
    # triangular diagonal-band mask: mtri[p, c] = 1 iff c >= p
    idx = np.arange(128)
    mtri = (idx[None, :] >= idx[:, None]).astype(NPBF16)

    in_maps = []
    for c in range(NCORES):
        h0, h1 = HPC * c, HPC * c + 1
        wqkv = np.concatenate(
            [
                Wq3[:, h0, :NOPE],
                Wq3[:, h1, :NOPE],
                Wq3[:, h0, NOPE:],
                Wq3[:, h1, NOPE:],
                Wkv_a,
            ],
            axis=1,
        )
        we_ = np.ascontiguousarray(
            W_embed[[h0, h1]] * kv_ln_w[None, :, None]).astype(NPBF16)
        wu_ = np.ascontiguousarray(
            np.concatenate([W_unembed[h0].T, W_unembed[h1].T], axis=1)
            * kv_ln_w[:, None]).astype(NPBF16)
        in_maps.append(
            {
                "xT": xT,
                "wqkv": np.ascontiguousarray(wqkv).astype(NPBF16),
                "we": we_,
                "wu": wu_,
                "wo0": np.ascontiguousarray(
                    Wo[h0 * VDIM: (h0 + 1) * VDIM]).astype(NPBF16),
                "wo1": np.ascontiguousarray(
                    Wo[h1 * VDIM: (h1 + 1) * VDIM]).astype(NPBF16),
                "mtri": mtri,
                "ones_col_d": np.ones((128, 1), NPBF16),
                "ones_row_d": np.ones((1, 128), NPBF16),
            }
        )
    return in_maps


def run(trace=False, tmpdir=None, **inputs):
    """Run the SPMD kernel; returns (full_output, BassKernelResults)."""
    inputs.pop("mask", None)  # causal structure is hardcoded
    nc = _get_nc()
    in_maps = _make_in_maps(**inputs)
    res = run_bass_kernel_spmd(
        nc, in_maps, core_ids=list(range(NCORES)), trace=trace, tmpdir=tmpdir
    )
    y = np.zeros((L, HID), dtype=np.float32)
    for c in range(NCORES):
        y += np.asarray(res.results[c]["y"], dtype=np.float32)
    return y.reshape(B, L, HID), res


def kernel(**inputs):
    y, _ = run(trace=False, **inputs)
    return y
